# revision 8
# baseline (speedup 1.0000x reference)
"""Causal self-attention + residual + LayerNorm fused Trainium2 kernel.

Problem: B=4, S=2048, D=1024, H=16 heads (hd=64), fp32 in/out.
    qkv = x @ in_proj_w.T + in_proj_b ; causal MHA ; out proj ; y = LN(x + attn_out)

Sharding (zero cross-core communication, 8 NeuronCores):
    core c -> batch b = c % 4, query-group g = c // 4.
    Causal zig-zag balance: g=0 owns query blocks [0:512) and [1536:2048),
    g=1 owns [512:1536). Every core computes full K/V for its batch,
    attention + out-proj + residual + LayerNorm for its own queries.

Layout/precision strategy:
  - All matmul operands are bf16 (fp32 PSUM accumulation): same PE rate as
    fp32r but half the DMA/SBUF and 2x DVE modes.
  - K^T / Q^T computed transposed (features on partitions); V computed
    natural (tokens on partitions) with an augmented ones-column so the
    probability matmul also produces the softmax denominator in PSUM row 64.
    V carries its projection bias (the bias rides through softmax).
  - K/V/Q projections and attention are emitted inside the same If branch so
    the Tile scheduler overlaps the PE-heavy projections with the ACT-bound
    (exp) attention stream: attention on k-tile i only depends on the K/V
    tiles already produced.
  - Scores for a head pair go to [128, 2, 512] PSUM blocks; exp is one
    [128,1024] ACT instruction per block per head. The causal mask is a bf16
    multiply on the exp'd block (last two blocks only); the final block only
    computes its live query half.
  - Out-projection is token-oriented (queries on partitions): LayerNorm
    reduces along the free axis via bn_stats/bn_aggr and normalizes with
    per-partition tensor_scalar ops. out_b is folded into the residual on
    the host; gamma/beta applied via broadcast tiles built once at init.
"""
import sys

if "/opt/trn_rl_repo" not in sys.path:
    sys.path.insert(0, "/opt/trn_rl_repo")

import numpy as np

B, S, D, H, HD = 4, 2048, 1024, 16, 64
P = 128
QT = 512                       # queries per q-tile
NQ = 1024                      # queries per core
NKT = S // P                   # 16 k-tiles per batch
DK = D // P                    # 8 contraction tiles over D
QSTART = {0: (0, 1536), 1: (512, 1024)}   # group -> q-tile start columns
NKS = {0: (4, 16), 1: (8, 12)}            # group -> per-q-tile k-tile counts

_cache = {}


def _build():
    import concourse.mybir as mybir
    import concourse.tile as tile
    from concourse import bacc
    from concourse.bass import ts
    from concourse.alu_op_type import AluOpType

    f32 = mybir.dt.float32
    bf16 = mybir.dt.bfloat16
    AF = mybir.ActivationFunctionType

    nc = bacc.Bacc("TRN2", target_bir_lowering=False, debug=False, num_devices=8)

    xt = nc.dram_tensor("xt", [D, S], bf16, kind="ExternalInput").ap()
    xrd = nc.dram_tensor("xrd", [NQ, D], f32, kind="ExternalInput").ap()
    wtb = nc.dram_tensor("wtb", [D, 3 * D], bf16, kind="ExternalInput").ap()
    wob = nc.dram_tensor("wob", [D, D], bf16, kind="ExternalInput").ap()
    mskd = nc.dram_tensor("mskd", [P, 4, QT], bf16, kind="ExternalInput").ap()
    bqd = nc.dram_tensor("bqd", [D], f32, kind="ExternalInput").ap()
    bkd = nc.dram_tensor("bkd", [D], f32, kind="ExternalInput").ap()
    bvd = nc.dram_tensor("bvd", [D], f32, kind="ExternalInput").ap()
    gamd = nc.dram_tensor("gamd", [D], bf16, kind="ExternalInput").ap()
    betd = nc.dram_tensor("betd", [D], bf16, kind="ExternalInput").ap()
    ytd = nc.dram_tensor("ytd", [NQ, D], f32, kind="ExternalOutput").ap()

    xt_r = xt.rearrange("(dk p) t -> p dk t", p=P)
    wq_src = wtb[:, 0:D].rearrange("(dk p) (f c) -> p dk f c", p=P, c=P)
    wk_src = wtb[:, D:2 * D].rearrange("(dk p) (f c) -> p dk f c", p=P, c=P)
    wv_src = wtb[:, 2 * D:3 * D].rearrange("(dk p) (g c) -> p dk g c", p=P, c=QT)
    wo_src = wob.rearrange("(dk p) f -> p dk f", p=P)

    with tile.TileContext(nc) as tc:
        with tc.tile_pool(name="persist", bufs=1) as pers:
            kt = pers.tile([P, DK, S], bf16)              # K^T      32 KB/part
            v = pers.tile([P, NKT, H, HD + 2], bf16)      # V aug    33 KB/part
            msk = pers.tile([P, 4, QT], bf16)
            bia = pers.tile([P, DK, 2], f32)              # bq bk per-partition
            gbc = pers.tile([P, D], bf16)                 # gamma broadcast
            bbc = pers.tile([P, D], bf16)                 # beta broadcast
            bvbc = pers.tile([P, 2, DK, HD], f32)         # V-bias broadcast
            eps_t = pers.tile([P, 1], f32)

            nc.vector.memset(eps_t[:], 1e-5)
            nc.sync.dma_start(msk[:], mskd[:])
            nc.sync.dma_start(bia[:, :, 0], bqd.rearrange("(f p) -> p f", p=P))
            nc.sync.dma_start(bia[:, :, 1], bkd.rearrange("(f p) -> p f", p=P))
            nc.vector.memset(v[:, :, :, HD:HD + 1], 1.0)

            with (
                tc.tile_pool(name="initrows", bufs=1) as irp,
                tc.tile_pool(name="wk", bufs=2) as wkp,
                tc.tile_pool(name="wv", bufs=1) as wvp,
                tc.tile_pool(name="wq", bufs=1) as wqp,
                tc.tile_pool(name="wot", bufs=1) as wotp,
                tc.tile_pool(name="xc", bufs=2) as xcp,
                tc.tile_pool(name="qt", bufs=2) as qtp,
                tc.tile_pool(name="cx", bufs=1) as cxp,
                tc.tile_pool(name="se", bufs=2) as sep,
                tc.tile_pool(name="scr", bufs=2) as scr,
                tc.tile_pool(name="yx", bufs=2) as yxp,
                tc.tile_pool(name="pp", bufs=2, space="PSUM") as pp,
                tc.tile_pool(name="s_ps", bufs=2, space="PSUM") as sps,
                tc.tile_pool(name="c_ps", bufs=2, space="PSUM") as cps,
            ):
                rowv = irp.tile([1, D], f32)
                rowg = irp.tile([1, D], bf16)
                rowb = irp.tile([1, D], bf16)
                nc.sync.dma_start(rowv[:], bvd.rearrange("(a d) -> a d", a=1))
                nc.sync.dma_start(rowg[:], gamd.rearrange("(a d) -> a d", a=1))
                nc.sync.dma_start(rowb[:], betd.rearrange("(a d) -> a d", a=1))
                nc.gpsimd.partition_broadcast(bvbc[:], rowv[:])
                nc.gpsimd.partition_broadcast(gbc[:], rowg[:])
                nc.gpsimd.partition_broadcast(bbc[:], rowb[:])

                wv = wvp.tile([P, DK, 2, QT], bf16)
                nc.sync.dma_start(wv[:], wv_src)
                wq = wqp.tile([P, DK, DK, P], bf16)
                nc.sync.dma_start(wq[:], wq_src)
                wot = wotp.tile([P, DK, D], bf16)
                nc.sync.dma_start(wot[:], wo_src)

                def kvproj():
                    for t in range(S // QT):
                        xc = xcp.tile([P, DK, QT], bf16, tag="xc")
                        nc.sync.dma_start(xc[:], xt_r[:, :, ts(t, QT)])
                        for f in range(DK):
                            wkt = wkp.tile([P, DK, P], bf16, tag="wk")
                            nc.sync.dma_start(wkt[:], wk_src[:, :, f, :])
                            ps = pp.tile([P, QT], f32, tag="pp")
                            for dk in range(DK):
                                nc.tensor.matmul(
                                    ps[:], wkt[:, dk, :], xc[:, dk, :],
                                    start=(dk == 0), stop=(dk == DK - 1),
                                )
                            nc.vector.tensor_scalar_add(
                                kt[:, f, ts(t, QT)], ps[:], bia[:, f, 1:2])
                        for q4 in range(4):
                            kti = 4 * t + q4
                            for fg in range(2):
                                ps = pp.tile([P, DK, HD], f32, tag="pp")
                                for dk in range(DK):
                                    nc.tensor.matmul(
                                        ps[:], xc[:, dk, ts(q4, P)],
                                        wv[:, dk, fg, :],
                                        start=(dk == 0), stop=(dk == DK - 1),
                                    )
                                nc.vector.tensor_add(
                                    v[:, kti, 8 * fg:8 * fg + 8, 0:HD],
                                    ps[:], bvbc[:, fg, :, :],
                                )

                def qproj(qpos, qtile):
                    xcq = xcp.tile([P, DK, QT], bf16, tag="xc")
                    nc.sync.dma_start(xcq[:], xt_r[:, :, qpos:qpos + QT])
                    for f in range(DK):
                        ps = pp.tile([P, QT], f32, tag="pp")
                        for dk in range(DK):
                            nc.tensor.matmul(
                                ps[:], wq[:, dk, f, :], xcq[:, dk, :],
                                start=(dk == 0), stop=(dk == DK - 1),
                            )
                        nc.vector.tensor_scalar_add(
                            qtile[:, f, :], ps[:], bia[:, f, 0:1])

                def attn(nk, qtile, ctxn):
                    nblk = nk // 2
                    for hp in range(H // 2):
                        cp0 = cps.tile([HD + 1, QT], f32, tag="c")
                        cp1 = cps.tile([HD + 1, QT], f32, tag="c")
                        for blk in range(nblk):
                            i0 = 2 * blk
                            # final diagonal block: only queries >= 256 have
                            # any live key; compute the live half only
                            qs_ = slice(QT // 2, QT) if blk == nblk - 1 \
                                else slice(0, QT)
                            sp0 = sps.tile([P, 2, QT], f32, tag="s")
                            sp1 = sps.tile([P, 2, QT], f32, tag="s")
                            for j in range(2):
                                nc.tensor.matmul(
                                    sp0[:, j, qs_], kt[0:HD, hp, ts(i0 + j, P)],
                                    qtile[0:HD, hp, qs_], start=True, stop=True,
                                )
                                nc.tensor.matmul(
                                    sp1[:, j, qs_], kt[HD:P, hp, ts(i0 + j, P)],
                                    qtile[HD:P, hp, qs_], start=True, stop=True,
                                )
                            se0 = sep.tile([P, 2, QT], bf16, tag="se")
                            se1 = sep.tile([P, 2, QT], bf16, tag="se")
                            nc.scalar.activation(
                                se0[:, :, qs_], sp0[:, :, qs_], AF.Exp, scale=0.125)
                            nc.scalar.activation(
                                se1[:, :, qs_], sp1[:, :, qs_], AF.Exp, scale=0.125)
                            if blk >= nblk - 2:
                                dd = 2 * blk - (nk - 4)
                                nc.vector.tensor_mul(
                                    se0[:, :, qs_], se0[:, :, qs_],
                                    msk[:, dd:dd + 2, qs_])
                                nc.vector.tensor_mul(
                                    se1[:, :, qs_], se1[:, :, qs_],
                                    msk[:, dd:dd + 2, qs_])
                            for j in range(2):
                                st = (blk == 0 and j == 0)
                                sp_ = (blk == nblk - 1 and j == 1)
                                nc.tensor.matmul(
                                    cp0[:, qs_], v[:, i0 + j, 2 * hp, 0:HD + 1],
                                    se0[:, j, qs_], start=st, stop=sp_,
                                )
                                nc.tensor.matmul(
                                    cp1[:, qs_], v[:, i0 + j, 2 * hp + 1, 0:HD + 1],
                                    se1[:, j, qs_], start=st, stop=sp_,
                                )
                        # both heads' denominators on partition 0 (engine
                        # APs may only start at quadrant partition bases)
                        den2 = scr.tile([1, 2, QT], f32, tag="den", bufs=1)
                        nc.vector.tensor_copy(den2[:, 0, :], cp0[HD:HD + 1, :])
                        nc.vector.tensor_copy(den2[:, 1, :], cp1[HD:HD + 1, :])
                        rec2 = scr.tile([1, 2, QT], f32, tag="rec", bufs=1)
                        rsc2 = scr.tile([1, 2, QT], f32, tag="rsc", bufs=1)
                        nc.vector.reciprocal_approx_accurate(
                            rec2[:], den2[:], rsc2[:])
                        bc0 = scr.tile([HD, QT], f32, tag="bc")
                        bc1 = scr.tile([HD, QT], f32, tag="bc")
                        nc.gpsimd.partition_broadcast(bc0[:], rec2[:, 0, :])
                        nc.gpsimd.partition_broadcast(bc1[:], rec2[:, 1, :])
                        nc.vector.tensor_mul(
                            ctxn[0:HD, hp, :], cp0[0:HD, :], bc0[:])
                        nc.vector.tensor_mul(
                            ctxn[HD:P, hp, :], cp1[0:HD, :], bc1[:])

                def outproj_ln(qt, ctxn):
                    for qs in range(4):
                        row0 = qt * QT + qs * P
                        xrt = yxp.tile([P, D], f32, tag="xr")
                        nc.sync.dma_start(xrt[:], xrd[row0:row0 + P, :])
                        for fh in range(2):
                            ps = pp.tile([P, QT], f32, tag="pp")
                            for dk in range(DK):
                                nc.tensor.matmul(
                                    ps[:], ctxn[:, dk, ts(qs, P)],
                                    wot[:, dk, ts(fh, QT)],
                                    start=(dk == 0), stop=(dk == DK - 1),
                                )
                            nc.vector.tensor_add(
                                xrt[:, ts(fh, QT)], ps[:], xrt[:, ts(fh, QT)])
                        st6 = scr.tile([P, 12], f32, tag="st6")
                        nc.vector.bn_stats(st6[:, 0:6], xrt[:, 0:QT])
                        nc.vector.bn_stats(st6[:, 6:12], xrt[:, QT:D])
                        mv = scr.tile([P, 2], f32, tag="mv")
                        nc.vector.bn_aggr(mv[:], st6[:])
                        sd = scr.tile([P, 1], f32, tag="sd")
                        nc.scalar.activation(
                            sd[:], mv[:, 1:2], AF.Sqrt, bias=eps_t[:])
                        rstd = scr.tile([P, 1], f32, tag="rstd")
                        rss = scr.tile([P, 1], f32, tag="rss")
                        nc.vector.reciprocal_approx_accurate(
                            rstd[:], sd[:], rss[:])
                        nc.vector.tensor_scalar(
                            xrt[:], xrt[:], mv[:, 0:1], rstd[:],
                            AluOpType.subtract, AluOpType.mult,
                        )
                        nc.vector.tensor_mul(xrt[:], xrt[:], gbc[:])
                        nc.vector.tensor_add(xrt[:], xrt[:], bbc[:])
                        nc.sync.dma_start(ytd[row0:row0 + P, :], xrt[:])

                def group(g):
                    qtile0 = qtp.tile([P, DK, QT], bf16, tag="qtile")
                    qproj(QSTART[g][0], qtile0)
                    kvproj()
                    for qt in range(2):
                        if qt == 0:
                            qtile = qtile0
                        else:
                            qtile = qtp.tile([P, DK, QT], bf16, tag="qtile")
                            qproj(QSTART[g][1], qtile)
                        ctxn = cxp.tile([P, DK, QT], bf16, tag="ctxn")
                        attn(NKS[g][qt], qtile, ctxn)
                        outproj_ln(qt, ctxn)

                pid = nc.partition_id()
                with tc.If(pid < 4) as cmp:
                    group(0)
                with cmp.Else():
                    group(1)
    nc.compile()
    return nc


def _get_nc():
    if "nc" not in _cache:
        _cache["nc"] = _build()
    return _cache["nc"]


def _prep(x, in_proj_w, in_proj_b, out_w, out_b, gamma, beta):
    from ml_dtypes import bfloat16

    x = np.asarray(x, np.float32)
    wtb = np.ascontiguousarray(np.asarray(in_proj_w, np.float32).T).astype(bfloat16)
    wob = np.ascontiguousarray(np.asarray(out_w, np.float32).T).astype(bfloat16)
    bqkv = np.asarray(in_proj_b, np.float32)
    bo = np.asarray(out_b, np.float32)
    gam = np.asarray(gamma, np.float32)
    bet = np.asarray(beta, np.float32)
    pp_, dd_, qq_ = np.arange(P)[:, None, None], np.arange(4)[None, :, None], \
        np.arange(QT)[None, None, :]
    maskd = (qq_ >= dd_ * P + pp_).astype(bfloat16)
    qcols = {
        0: np.r_[0:QT, 3 * QT:4 * QT],
        1: np.r_[QT:3 * QT],
    }
    in_maps = []
    for c in range(8):
        b, g = c % 4, c // 4
        xtb = np.ascontiguousarray(x[b].T).astype(bfloat16)
        xr = np.ascontiguousarray(x[b][qcols[g]]) + bo[None, :]
        in_maps.append({
            "xt": xtb,
            "xrd": xr.astype(np.float32),
            "wtb": wtb,
            "wob": wob,
            "mskd": maskd,
            "bqd": bqkv[0:D], "bkd": bqkv[D:2 * D], "bvd": bqkv[2 * D:3 * D],
            "gamd": gam.astype(bfloat16), "betd": bet.astype(bfloat16),
        })
    return in_maps, qcols


def _run(in_maps, trace=False, **kw):
    from concourse.bass_utils import run_bass_kernel_spmd

    return run_bass_kernel_spmd(_get_nc(), in_maps, list(range(8)), trace=trace, **kw)


def kernel(x, in_proj_w, in_proj_b, out_w, out_b, gamma, beta):
    in_maps, qcols = _prep(x, in_proj_w, in_proj_b, out_w, out_b, gamma, beta)
    res = _run(in_maps)
    out = np.empty((B, S, D), np.float32)
    for c in range(8):
        out[c % 4, qcols[c // 4]] = res.results[c]["ytd"]
    return out


# revision 11
# speedup vs baseline: 1.1694x; 1.1694x over previous
"""Causal self-attention + residual + LayerNorm fused Trainium2 kernel.

Problem: B=4, S=2048, D=1024, H=16 heads (hd=64), fp32 in/out.
    qkv = x @ in_proj_w.T + in_proj_b ; causal MHA ; out proj ; y = LN(x + attn_out)

Sharding (zero cross-core communication, 8 NeuronCores):
    core c -> batch b = c % 4, query-group g = c // 4.
    Causal zig-zag balance: g=0 owns query blocks [0:512) and [1536:2048),
    g=1 owns [512:1536). Every core computes full K/V for its batch,
    attention + out-proj + residual + LayerNorm for its own queries.

Layout/precision strategy:
  - All matmul operands are bf16 (fp32 PSUM accumulation): same PE rate as
    fp32r but half the DMA/SBUF and 2x DVE modes.
  - K^T / Q^T computed transposed (features on partitions); V computed
    natural (tokens on partitions) with an augmented ones-column so the
    probability matmul also produces the softmax denominator in PSUM row 64.
    V carries its projection bias (the bias rides through softmax).
  - K/V/Q projections and attention are emitted inside the same If branch so
    the Tile scheduler overlaps the PE-heavy projections with the ACT-bound
    (exp) attention stream: attention on k-tile i only depends on the K/V
    tiles already produced.
  - Scores for a head pair go to [128, 2, 512] PSUM blocks; exp is one
    [128,1024] ACT instruction per block per head. The causal mask is a bf16
    multiply on the exp'd block (last two blocks only); the final block only
    computes its live query half.
  - Out-projection is token-oriented (queries on partitions): LayerNorm
    reduces along the free axis via bn_stats/bn_aggr and normalizes with
    per-partition tensor_scalar ops. out_b is folded into the residual on
    the host; gamma/beta applied via broadcast tiles built once at init.
"""
import sys

if "/opt/trn_rl_repo" not in sys.path:
    sys.path.insert(0, "/opt/trn_rl_repo")

import numpy as np

B, S, D, H, HD = 4, 2048, 1024, 16, 64
P = 128
QT = 512                       # queries per q-tile
NQ = 1024                      # queries per core
NKT = S // P                   # 16 k-tiles per batch
DK = D // P                    # 8 contraction tiles over D
QSTART = {0: (0, 1536), 1: (512, 1024)}   # group -> q-tile start columns
NKS = {0: (4, 16), 1: (8, 12)}            # group -> per-q-tile k-tile counts

_cache = {}


def _build():
    import concourse.mybir as mybir
    import concourse.tile as tile
    from concourse import bacc
    from concourse.bass import ts
    from concourse.alu_op_type import AluOpType

    f32 = mybir.dt.float32
    bf16 = mybir.dt.bfloat16
    AF = mybir.ActivationFunctionType

    nc = bacc.Bacc("TRN2", target_bir_lowering=False, debug=False, num_devices=8)

    xt = nc.dram_tensor("xt", [D, S], bf16, kind="ExternalInput").ap()
    xrd = nc.dram_tensor("xrd", [NQ, D], f32, kind="ExternalInput").ap()
    wtb = nc.dram_tensor("wtb", [D, 3 * D], bf16, kind="ExternalInput").ap()
    wob = nc.dram_tensor("wob", [D, D], bf16, kind="ExternalInput").ap()
    mskd = nc.dram_tensor("mskd", [P, 4, QT], bf16, kind="ExternalInput").ap()
    bqd = nc.dram_tensor("bqd", [D], f32, kind="ExternalInput").ap()
    bkd = nc.dram_tensor("bkd", [D], f32, kind="ExternalInput").ap()
    bvd = nc.dram_tensor("bvd", [D], f32, kind="ExternalInput").ap()
    gamd = nc.dram_tensor("gamd", [D], bf16, kind="ExternalInput").ap()
    betd = nc.dram_tensor("betd", [D], bf16, kind="ExternalInput").ap()
    ytd = nc.dram_tensor("ytd", [NQ, D], f32, kind="ExternalOutput").ap()

    xt_r = xt.rearrange("(dk p) t -> p dk t", p=P)
    wq_src = wtb[:, 0:D].rearrange("(dk p) (f c) -> p dk f c", p=P, c=P)
    wk_src = wtb[:, D:2 * D].rearrange("(dk p) (f c) -> p dk f c", p=P, c=P)
    wv_src = wtb[:, 2 * D:3 * D].rearrange("(dk p) (g c) -> p dk g c", p=P, c=QT)
    wo_src = wob.rearrange("(dk p) f -> p dk f", p=P)

    with tile.TileContext(nc) as tc:
        with tc.tile_pool(name="persist", bufs=1) as pers:
            kt = pers.tile([P, DK, S], bf16)              # K^T      32 KB/part
            v = pers.tile([P, NKT, H, HD + 2], bf16)      # V aug    33 KB/part
            msk = pers.tile([P, 4, QT], bf16)
            bia = pers.tile([P, DK, 2], f32)              # bq bk per-partition
            gbc = pers.tile([P, D], bf16)                 # gamma broadcast
            bbc = pers.tile([P, D], bf16)                 # beta broadcast
            bvbc = pers.tile([P, 2, DK, HD], f32)         # V-bias broadcast
            eps_t = pers.tile([P, 1], f32)

            nc.vector.memset(eps_t[:], 1e-5)
            nc.sync.dma_start(msk[:], mskd[:])
            nc.sync.dma_start(bia[:, :, 0], bqd.rearrange("(f p) -> p f", p=P))
            nc.sync.dma_start(bia[:, :, 1], bkd.rearrange("(f p) -> p f", p=P))
            nc.vector.memset(v[:, :, :, HD:HD + 1], 1.0)

            with (
                tc.tile_pool(name="initrows", bufs=1) as irp,
                tc.tile_pool(name="wk", bufs=2) as wkp,
                tc.tile_pool(name="wv", bufs=1) as wvp,
                tc.tile_pool(name="wot", bufs=1) as wotp,
                tc.tile_pool(name="xc", bufs=2) as xcp,
                tc.tile_pool(name="qt", bufs=2) as qtp,
                tc.tile_pool(name="cx", bufs=2) as cxp,
                tc.tile_pool(name="se", bufs=2) as sep,
                tc.tile_pool(name="scr", bufs=2) as scr,
                tc.tile_pool(name="yx", bufs=2) as yxp,
                tc.tile_pool(name="pp", bufs=2, space="PSUM") as pp,
                tc.tile_pool(name="s_ps", bufs=2, space="PSUM") as sps,
                tc.tile_pool(name="c_ps", bufs=2, space="PSUM") as cps,
            ):
                rowv = irp.tile([1, D], f32)
                rowg = irp.tile([1, D], bf16)
                rowb = irp.tile([1, D], bf16)
                nc.sync.dma_start(rowv[:], bvd.rearrange("(a d) -> a d", a=1))
                nc.sync.dma_start(rowg[:], gamd.rearrange("(a d) -> a d", a=1))
                nc.sync.dma_start(rowb[:], betd.rearrange("(a d) -> a d", a=1))
                nc.gpsimd.partition_broadcast(bvbc[:], rowv[:])
                nc.gpsimd.partition_broadcast(gbc[:], rowg[:])
                nc.gpsimd.partition_broadcast(bbc[:], rowb[:])

                wv = wvp.tile([P, DK, 2, QT], bf16)
                nc.sync.dma_start(wv[:], wv_src)
                wot = wotp.tile([P, DK, D], bf16)
                nc.sync.dma_start(wot[:], wo_src)

                def kvproj():
                    for t in range(S // QT):
                        xc = xcp.tile([P, DK, QT], bf16, tag="xc")
                        nc.sync.dma_start(xc[:], xt_r[:, :, ts(t, QT)])
                        for f in range(DK):
                            wkt = wkp.tile([P, DK, P], bf16, tag="wk")
                            nc.sync.dma_start(wkt[:], wk_src[:, :, f, :])
                            ps = pp.tile([P, QT], f32, tag="pp")
                            for dk in range(DK):
                                nc.tensor.matmul(
                                    ps[:], wkt[:, dk, :], xc[:, dk, :],
                                    start=(dk == 0), stop=(dk == DK - 1),
                                )
                            nc.vector.tensor_scalar_add(
                                kt[:, f, ts(t, QT)], ps[:], bia[:, f, 1:2])
                        for q4 in range(4):
                            kti = 4 * t + q4
                            for fg in range(2):
                                ps = pp.tile([P, DK, HD], f32, tag="pp")
                                for dk in range(DK):
                                    nc.tensor.matmul(
                                        ps[:], xc[:, dk, ts(q4, P)],
                                        wv[:, dk, fg, :],
                                        start=(dk == 0), stop=(dk == DK - 1),
                                    )
                                nc.vector.tensor_add(
                                    v[:, kti, 8 * fg:8 * fg + 8, 0:HD],
                                    ps[:], bvbc[:, fg, :, :],
                                )

                def qproj(qpos, qtile, wqp):
                    xcq = xcp.tile([P, DK, QT], bf16, tag="xc")
                    nc.sync.dma_start(xcq[:], xt_r[:, :, qpos:qpos + QT])
                    for f in range(DK):
                        wq = wqp.tile([P, DK, P], bf16, tag="wq")
                        nc.sync.dma_start(wq[:], wq_src[:, :, f, :])
                        ps = pp.tile([P, QT], f32, tag="pp")
                        for dk in range(DK):
                            nc.tensor.matmul(
                                ps[:], wq[:, dk, :], xcq[:, dk, :],
                                start=(dk == 0), stop=(dk == DK - 1),
                            )
                        nc.vector.tensor_scalar_add(
                            qtile[:, f, :], ps[:], bia[:, f, 0:1])

                def attn_pair(nk, qtile, ctxn, hp):
                    nblk = nk // 2
                    if True:
                        cp0 = cps.tile([HD + 1, QT], f32, tag="c")
                        cp1 = cps.tile([HD + 1, QT], f32, tag="c")
                        for blk in range(nblk):
                            i0 = 2 * blk
                            # final diagonal block: only queries >= 256 have
                            # any live key; compute the live half only
                            qs_ = slice(QT // 2, QT) if blk == nblk - 1 \
                                else slice(0, QT)
                            sp0 = sps.tile([P, 2, QT], f32, tag="s")
                            sp1 = sps.tile([P, 2, QT], f32, tag="s")
                            for j in range(2):
                                nc.tensor.matmul(
                                    sp0[:, j, qs_], kt[0:HD, hp, ts(i0 + j, P)],
                                    qtile[0:HD, hp, qs_], start=True, stop=True,
                                )
                                nc.tensor.matmul(
                                    sp1[:, j, qs_], kt[HD:P, hp, ts(i0 + j, P)],
                                    qtile[HD:P, hp, qs_], start=True, stop=True,
                                )
                            se0 = sep.tile([P, 2, QT], bf16, tag="se")
                            se1 = sep.tile([P, 2, QT], bf16, tag="se")
                            nc.scalar.activation(
                                se0[:, :, qs_], sp0[:, :, qs_], AF.Exp, scale=0.125)
                            nc.scalar.activation(
                                se1[:, :, qs_], sp1[:, :, qs_], AF.Exp, scale=0.125)
                            if blk >= nblk - 2:
                                dd = 2 * blk - (nk - 4)
                                nc.vector.tensor_mul(
                                    se0[:, :, qs_], se0[:, :, qs_],
                                    msk[:, dd:dd + 2, qs_])
                                nc.vector.tensor_mul(
                                    se1[:, :, qs_], se1[:, :, qs_],
                                    msk[:, dd:dd + 2, qs_])
                            for j in range(2):
                                st = (blk == 0 and j == 0)
                                sp_ = (blk == nblk - 1 and j == 1)
                                nc.tensor.matmul(
                                    cp0[:, qs_], v[:, i0 + j, 2 * hp, 0:HD + 1],
                                    se0[:, j, qs_], start=st, stop=sp_,
                                )
                                nc.tensor.matmul(
                                    cp1[:, qs_], v[:, i0 + j, 2 * hp + 1, 0:HD + 1],
                                    se1[:, j, qs_], start=st, stop=sp_,
                                )
                        # both heads' denominators on partition 0 (engine
                        # APs may only start at quadrant partition bases)
                        den2 = scr.tile([1, 2, QT], f32, tag="den", bufs=1)
                        nc.vector.tensor_copy(den2[:, 0, :], cp0[HD:HD + 1, :])
                        nc.vector.tensor_copy(den2[:, 1, :], cp1[HD:HD + 1, :])
                        rec2 = scr.tile([1, 2, QT], f32, tag="rec", bufs=1)
                        rsc2 = scr.tile([1, 2, QT], f32, tag="rsc", bufs=1)
                        nc.vector.reciprocal_approx_accurate(
                            rec2[:], den2[:], rsc2[:])
                        bc2 = scr.tile([HD, 2, QT], f32, tag="bc")
                        nc.gpsimd.partition_broadcast(bc2[:], rec2[:])
                        nc.vector.tensor_mul(
                            ctxn[0:HD, hp, :], cp0[0:HD, :], bc2[:, 0, :])
                        nc.vector.tensor_mul(
                            ctxn[HD:P, hp, :], cp1[0:HD, :], bc2[:, 1, :])

                def outproj_ln(qt, ctxn):
                    for qs in range(4):
                        row0 = qt * QT + qs * P
                        xrt = yxp.tile([P, D], f32, tag="xr")
                        nc.sync.dma_start(xrt[:], xrd[row0:row0 + P, :])
                        for fh in range(2):
                            ps = pp.tile([P, QT], f32, tag="pp")
                            for dk in range(DK):
                                nc.tensor.matmul(
                                    ps[:], ctxn[:, dk, ts(qs, P)],
                                    wot[:, dk, ts(fh, QT)],
                                    start=(dk == 0), stop=(dk == DK - 1),
                                )
                            nc.vector.tensor_add(
                                xrt[:, ts(fh, QT)], ps[:], xrt[:, ts(fh, QT)])
                        st6 = scr.tile([P, 12], f32, tag="st6")
                        nc.vector.bn_stats(st6[:, 0:6], xrt[:, 0:QT])
                        nc.vector.bn_stats(st6[:, 6:12], xrt[:, QT:D])
                        mv = scr.tile([P, 2], f32, tag="mv")
                        nc.vector.bn_aggr(mv[:], st6[:])
                        sd = scr.tile([P, 1], f32, tag="sd")
                        nc.scalar.activation(
                            sd[:], mv[:, 1:2], AF.Sqrt, bias=eps_t[:])
                        rstd = scr.tile([P, 1], f32, tag="rstd")
                        rss = scr.tile([P, 1], f32, tag="rss")
                        nc.vector.reciprocal_approx_accurate(
                            rstd[:], sd[:], rss[:])
                        nc.vector.tensor_scalar(
                            xrt[:], xrt[:], mv[:, 0:1], rstd[:],
                            AluOpType.subtract, AluOpType.mult,
                        )
                        nc.vector.tensor_mul(xrt[:], xrt[:], gbc[:])
                        nc.vector.tensor_add(xrt[:], xrt[:], bbc[:])
                        nc.sync.dma_start(ytd[row0:row0 + P, :], xrt[:])

                def group(g):
                    qtile0 = qtp.tile([P, DK, QT], bf16, tag="qtile")
                    qtile1 = qtp.tile([P, DK, QT], bf16, tag="qtile")
                    with tc.tile_pool(name="wq", bufs=2) as wqp:
                        qproj(QSTART[g][0], qtile0, wqp)
                        qproj(QSTART[g][1], qtile1, wqp)
                        kvproj()
                    ctxn0 = cxp.tile([P, DK, QT], bf16, tag="ctxn")
                    ctxn1 = cxp.tile([P, DK, QT], bf16, tag="ctxn")
                    for hp in range(H // 2):
                        attn_pair(NKS[g][0], qtile0, ctxn0, hp)
                        attn_pair(NKS[g][1], qtile1, ctxn1, hp)
                    outproj_ln(0, ctxn0)
                    outproj_ln(1, ctxn1)

                pid = nc.partition_id()
                with tc.If(pid < 4) as cmp:
                    group(0)
                with cmp.Else():
                    group(1)
    nc.compile()
    return nc


def _get_nc():
    if "nc" not in _cache:
        _cache["nc"] = _build()
    return _cache["nc"]


def _prep(x, in_proj_w, in_proj_b, out_w, out_b, gamma, beta):
    from ml_dtypes import bfloat16

    x = np.asarray(x, np.float32)
    wtb = np.ascontiguousarray(np.asarray(in_proj_w, np.float32).T).astype(bfloat16)
    wob = np.ascontiguousarray(np.asarray(out_w, np.float32).T).astype(bfloat16)
    bqkv = np.asarray(in_proj_b, np.float32)
    bo = np.asarray(out_b, np.float32)
    gam = np.asarray(gamma, np.float32)
    bet = np.asarray(beta, np.float32)
    pp_, dd_, qq_ = np.arange(P)[:, None, None], np.arange(4)[None, :, None], \
        np.arange(QT)[None, None, :]
    maskd = (qq_ >= dd_ * P + pp_).astype(bfloat16)
    qcols = {
        0: np.r_[0:QT, 3 * QT:4 * QT],
        1: np.r_[QT:3 * QT],
    }
    in_maps = []
    for c in range(8):
        b, g = c % 4, c // 4
        xtb = np.ascontiguousarray(x[b].T).astype(bfloat16)
        xr = np.ascontiguousarray(x[b][qcols[g]]) + bo[None, :]
        in_maps.append({
            "xt": xtb,
            "xrd": xr.astype(np.float32),
            "wtb": wtb,
            "wob": wob,
            "mskd": maskd,
            "bqd": bqkv[0:D], "bkd": bqkv[D:2 * D], "bvd": bqkv[2 * D:3 * D],
            "gamd": gam.astype(bfloat16), "betd": bet.astype(bfloat16),
        })
    return in_maps, qcols


def _run(in_maps, trace=False, **kw):
    from concourse.bass_utils import run_bass_kernel_spmd

    return run_bass_kernel_spmd(_get_nc(), in_maps, list(range(8)), trace=trace, **kw)


def kernel(x, in_proj_w, in_proj_b, out_w, out_b, gamma, beta):
    in_maps, qcols = _prep(x, in_proj_w, in_proj_b, out_w, out_b, gamma, beta)
    res = _run(in_maps)
    out = np.empty((B, S, D), np.float32)
    for c in range(8):
        out[c % 4, qcols[c // 4]] = res.results[c]["ytd"]
    return out


# revision 15
# speedup vs baseline: 1.2227x; 1.0456x over previous
"""Causal self-attention + residual + LayerNorm fused Trainium2 kernel.

Problem: B=4, S=2048, D=1024, H=16 heads (hd=64), fp32 in/out.
    qkv = x @ in_proj_w.T + in_proj_b ; causal MHA ; out proj ; y = LN(x + attn_out)

Sharding (zero cross-core communication, 8 NeuronCores):
    core c -> batch b = c % 4, query-group g = c // 4.
    Causal zig-zag balance: g=0 owns query blocks [0:512) and [1536:2048),
    g=1 owns [512:1536). Every core computes full K/V for its batch,
    attention + out-proj + residual + LayerNorm for its own queries.

Layout/precision strategy:
  - All matmul operands are bf16 (fp32 PSUM accumulation): same PE rate as
    fp32r but half the DMA/SBUF and 2x DVE modes.
  - K^T / Q^T computed transposed (features on partitions); V computed
    natural (tokens on partitions) with an augmented ones-column so the
    probability matmul also produces the softmax denominator in PSUM row 64.
    V carries its projection bias (the bias rides through softmax).
  - K/V/Q projections and attention are emitted inside the same If branch so
    the Tile scheduler overlaps the PE-heavy projections with the ACT-bound
    (exp) attention stream: attention on k-tile i only depends on the K/V
    tiles already produced.
  - Scores for a head pair go to [128, 2, 512] PSUM blocks; exp is one
    [128,1024] ACT instruction per block per head. The causal mask is a bf16
    multiply on the exp'd block (last two blocks only); the final block only
    computes its live query half.
  - Out-projection is token-oriented (queries on partitions): LayerNorm
    reduces along the free axis via bn_stats/bn_aggr and normalizes with
    per-partition tensor_scalar ops. out_b is folded into the residual on
    the host; gamma/beta applied via broadcast tiles built once at init.
"""
import sys

if "/opt/trn_rl_repo" not in sys.path:
    sys.path.insert(0, "/opt/trn_rl_repo")

import numpy as np

B, S, D, H, HD = 4, 2048, 1024, 16, 64
P = 128
QT = 512                       # queries per q-tile
NQ = 1024                      # queries per core
NKT = S // P                   # 16 k-tiles per batch
DK = D // P                    # 8 contraction tiles over D
QSTART = {0: (0, 1536), 1: (512, 1024)}   # group -> q-tile start columns
NKS = {0: (4, 16), 1: (8, 12)}            # group -> per-q-tile k-tile counts

_cache = {}


def _build():
    import concourse.mybir as mybir
    import concourse.tile as tile
    from concourse import bacc
    from concourse.bass import ts
    from concourse.alu_op_type import AluOpType

    f32 = mybir.dt.float32
    bf16 = mybir.dt.bfloat16
    AF = mybir.ActivationFunctionType

    nc = bacc.Bacc("TRN2", target_bir_lowering=False, debug=False, num_devices=8)

    xt = nc.dram_tensor("xt", [D, S], bf16, kind="ExternalInput").ap()
    xrd = nc.dram_tensor("xrd", [NQ, D], f32, kind="ExternalInput").ap()
    wtb = nc.dram_tensor("wtb", [D, 3 * D], bf16, kind="ExternalInput").ap()
    wob = nc.dram_tensor("wob", [D, D], bf16, kind="ExternalInput").ap()
    mskd = nc.dram_tensor("mskd", [P, 4, QT], bf16, kind="ExternalInput").ap()
    bqd = nc.dram_tensor("bqd", [D], f32, kind="ExternalInput").ap()
    bkd = nc.dram_tensor("bkd", [D], f32, kind="ExternalInput").ap()
    bvd = nc.dram_tensor("bvd", [D], bf16, kind="ExternalInput").ap()
    gamd = nc.dram_tensor("gamd", [D], f32, kind="ExternalInput").ap()
    betd = nc.dram_tensor("betd", [D], f32, kind="ExternalInput").ap()
    ytd = nc.dram_tensor("ytd", [NQ, D], f32, kind="ExternalOutput").ap()

    xt_r = xt.rearrange("(dk p) t -> p dk t", p=P)
    wq_src = wtb[:, 0:D].rearrange("(dk p) (f c) -> p dk f c", p=P, c=P)
    wk_src = wtb[:, D:2 * D].rearrange("(dk p) (f c) -> p dk f c", p=P, c=P)
    wv_src = wtb[:, 2 * D:3 * D].rearrange("(dk p) (g c) -> p dk g c", p=P, c=QT)
    wo_src = wob.rearrange("(dk p) f -> p dk f", p=P)

    with tile.TileContext(nc) as tc:
        with tc.tile_pool(name="persist", bufs=1) as pers:
            kt = pers.tile([P, DK, S], bf16)              # K^T      32 KB/part
            v = pers.tile([P, NKT, H, HD + 2], bf16)      # V aug    33 KB/part
            msk = pers.tile([P, 4, QT], bf16)
            bia = pers.tile([P, DK, 2], f32)              # bq bk per-partition
            gbc = pers.tile([P, D], f32)                 # gamma broadcast
            bbc = pers.tile([P, D], f32)                 # beta broadcast
            bvbc = pers.tile([P, 2, DK, HD], bf16)         # V-bias broadcast
            eps_t = pers.tile([P, 1], f32)

            nc.vector.memset(eps_t[:], 1e-5)
            nc.sync.dma_start(msk[:], mskd[:])
            nc.sync.dma_start(bia[:, :, 0], bqd.rearrange("(f p) -> p f", p=P))
            nc.sync.dma_start(bia[:, :, 1], bkd.rearrange("(f p) -> p f", p=P))
            nc.vector.memset(v[:, :, :, HD:HD + 1], 1.0)

            with (
                tc.tile_pool(name="initrows", bufs=1) as irp,
                tc.tile_pool(name="wk", bufs=2) as wkp,
                tc.tile_pool(name="wv", bufs=1) as wvp,
                tc.tile_pool(name="wot", bufs=1) as wotp,
                tc.tile_pool(name="xc", bufs=2) as xcp,
                tc.tile_pool(name="qt", bufs=2) as qtp,
                tc.tile_pool(name="cx", bufs=2) as cxp,
                tc.tile_pool(name="se", bufs=2) as sep,
                tc.tile_pool(name="scr", bufs=2) as scr,
                tc.tile_pool(name="yx", bufs=2) as yxp,
                tc.tile_pool(name="pp", bufs=2, space="PSUM") as pp,
                tc.tile_pool(name="s_ps", bufs=2, space="PSUM") as sps,
                tc.tile_pool(name="c_ps", bufs=2, space="PSUM") as cps,
            ):
                rowv = irp.tile([1, D], bf16)
                rowg = irp.tile([1, D], f32)
                rowb = irp.tile([1, D], f32)
                nc.sync.dma_start(rowv[:], bvd.rearrange("(a d) -> a d", a=1))
                nc.sync.dma_start(rowg[:], gamd.rearrange("(a d) -> a d", a=1))
                nc.sync.dma_start(rowb[:], betd.rearrange("(a d) -> a d", a=1))
                nc.gpsimd.partition_broadcast(bvbc[:], rowv[:])
                nc.gpsimd.partition_broadcast(gbc[:], rowg[:])
                nc.gpsimd.partition_broadcast(bbc[:], rowb[:])

                wv = wvp.tile([P, DK, 2, QT], bf16)
                wot = wotp.tile([P, DK, D], bf16)

                def kvproj(trange):
                    for t in trange:
                        xc = xcp.tile([P, DK, QT], bf16, tag="xc")
                        nc.sync.dma_start(xc[:], xt_r[:, :, ts(t, QT)])
                        for f in range(DK):
                            wkt = wkp.tile([P, DK, P], bf16, tag="wk")
                            nc.sync.dma_start(wkt[:], wk_src[:, :, f, :])
                            ps = pp.tile([P, QT], f32, tag="pp")
                            for dk in range(DK):
                                nc.tensor.matmul(
                                    ps[:], wkt[:, dk, :], xc[:, dk, :],
                                    start=(dk == 0), stop=(dk == DK - 1),
                                )
                            nc.vector.tensor_scalar_add(
                                kt[:, f, ts(t, QT)], ps[:], bia[:, f, 1:2])
                        for q4 in range(4):
                            kti = 4 * t + q4
                            for fg in range(2):
                                ps = pp.tile([P, DK, HD], f32, tag="pp")
                                for dk in range(DK):
                                    nc.tensor.matmul(
                                        ps[:], xc[:, dk, ts(q4, P)],
                                        wv[:, dk, fg, :],
                                        start=(dk == 0), stop=(dk == DK - 1),
                                    )
                                nc.vector.tensor_add(
                                    v[:, kti, 8 * fg:8 * fg + 8, 0:HD],
                                    ps[:], bvbc[:, fg, :, :],
                                )

                def qproj(qpos, qtile, wqp):
                    xcq = xcp.tile([P, DK, QT], bf16, tag="xc")
                    nc.sync.dma_start(xcq[:], xt_r[:, :, qpos:qpos + QT])
                    for f in range(DK):
                        wq = wqp.tile([P, DK, P], bf16, tag="wq")
                        nc.sync.dma_start(wq[:], wq_src[:, :, f, :])
                        ps = pp.tile([P, QT], f32, tag="pp")
                        for dk in range(DK):
                            nc.tensor.matmul(
                                ps[:], wq[:, dk, :], xcq[:, dk, :],
                                start=(dk == 0), stop=(dk == DK - 1),
                            )
                        nc.vector.tensor_scalar_add(
                            qtile[:, f, :], ps[:], bia[:, f, 0:1])

                def attn_pair(nk, qtile, ctxn, hp):
                    nblk = nk // 2
                    if True:
                        cp0 = cps.tile([HD + 1, QT], f32, tag="c")
                        cp1 = cps.tile([HD + 1, QT], f32, tag="c")
                        for blk in range(nblk):
                            i0 = 2 * blk
                            # final diagonal block: only queries >= 256 have
                            # any live key; compute the live half only
                            qs_ = slice(QT // 2, QT) if blk == nblk - 1 \
                                else slice(0, QT)
                            sp0 = sps.tile([P, 2, QT], f32, tag="s")
                            sp1 = sps.tile([P, 2, QT], f32, tag="s")
                            for j in range(2):
                                nc.tensor.matmul(
                                    sp0[:, j, qs_], kt[0:HD, hp, ts(i0 + j, P)],
                                    qtile[0:HD, hp, qs_], start=True, stop=True,
                                )
                                nc.tensor.matmul(
                                    sp1[:, j, qs_], kt[HD:P, hp, ts(i0 + j, P)],
                                    qtile[HD:P, hp, qs_], start=True, stop=True,
                                )
                            se0 = sep.tile([P, 2, QT], bf16, tag="se")
                            se1 = sep.tile([P, 2, QT], bf16, tag="se")
                            nc.scalar.activation(
                                se0[:, :, qs_], sp0[:, :, qs_], AF.Exp, scale=0.125)
                            nc.scalar.activation(
                                se1[:, :, qs_], sp1[:, :, qs_], AF.Exp, scale=0.125)
                            if blk >= nblk - 2:
                                dd = 2 * blk - (nk - 4)
                                nc.vector.tensor_mul(
                                    se0[:, :, qs_], se0[:, :, qs_],
                                    msk[:, dd:dd + 2, qs_])
                                nc.vector.tensor_mul(
                                    se1[:, :, qs_], se1[:, :, qs_],
                                    msk[:, dd:dd + 2, qs_])
                            for j in range(2):
                                st = (blk == 0 and j == 0)
                                sp_ = (blk == nblk - 1 and j == 1)
                                nc.tensor.matmul(
                                    cp0[:, qs_], v[:, i0 + j, 2 * hp, 0:HD + 1],
                                    se0[:, j, qs_], start=st, stop=sp_,
                                )
                                nc.tensor.matmul(
                                    cp1[:, qs_], v[:, i0 + j, 2 * hp + 1, 0:HD + 1],
                                    se1[:, j, qs_], start=st, stop=sp_,
                                )
                        # both heads' denominators on partition 0 (engine
                        # APs may only start at quadrant partition bases)
                        den2 = scr.tile([1, 2, QT], f32, tag="den", bufs=1)
                        nc.vector.tensor_copy(den2[:, 0, :], cp0[HD:HD + 1, :])
                        nc.vector.tensor_copy(den2[:, 1, :], cp1[HD:HD + 1, :])
                        rec2 = scr.tile([1, 2, QT], f32, tag="rec", bufs=1)
                        rsc2 = scr.tile([1, 2, QT], f32, tag="bc", bufs=1)
                        nc.vector.reciprocal_approx_accurate(
                            rec2[:], den2[:], rsc2[:])
                        bc2 = scr.tile([HD, 2, QT], f32, tag="bc", bufs=1)
                        nc.gpsimd.partition_broadcast(bc2[:], rec2[:])
                        nc.vector.tensor_mul(
                            ctxn[0:HD, hp, :], cp0[0:HD, :], bc2[:, 0, :])
                        nc.vector.tensor_mul(
                            ctxn[HD:P, hp, :], cp1[0:HD, :], bc2[:, 1, :])

                def outproj_ln(qt, ctxn):
                    for qs in range(4):
                        row0 = qt * QT + qs * P
                        xrt = yxp.tile([P, D], f32, tag="xr")
                        nc.sync.dma_start(xrt[:], xrd[row0:row0 + P, :])
                        for fh in range(2):
                            ps = pp.tile([P, QT], f32, tag="pp")
                            for dk in range(DK):
                                nc.tensor.matmul(
                                    ps[:], ctxn[:, dk, ts(qs, P)],
                                    wot[:, dk, ts(fh, QT)],
                                    start=(dk == 0), stop=(dk == DK - 1),
                                )
                            nc.vector.tensor_add(
                                xrt[:, ts(fh, QT)], ps[:], xrt[:, ts(fh, QT)])
                        st6 = scr.tile([P, 12], f32, tag="st6")
                        nc.vector.bn_stats(st6[:, 0:6], xrt[:, 0:QT])
                        nc.vector.bn_stats(st6[:, 6:12], xrt[:, QT:D])
                        mv = scr.tile([P, 2], f32, tag="mv")
                        nc.vector.bn_aggr(mv[:], st6[:])
                        sd = scr.tile([P, 1], f32, tag="sd")
                        nc.scalar.activation(
                            sd[:], mv[:, 1:2], AF.Sqrt, bias=eps_t[:])
                        rstd = scr.tile([P, 1], f32, tag="rstd")
                        rss = scr.tile([P, 1], f32, tag="rss")
                        nc.vector.reciprocal_approx_accurate(
                            rstd[:], sd[:], rss[:])
                        nc.vector.tensor_scalar(
                            xrt[:], xrt[:], mv[:, 0:1], rstd[:],
                            AluOpType.subtract, AluOpType.mult,
                        )
                        nc.gpsimd.tensor_mul(xrt[:], xrt[:], gbc[:])
                        nc.gpsimd.tensor_add(xrt[:], xrt[:], bbc[:])
                        nc.sync.dma_start(ytd[row0:row0 + P, :], xrt[:])

                def group(g):
                    qtile0 = qtp.tile([P, DK, QT], bf16, tag="qtile")
                    qtile1 = qtp.tile([P, DK, QT], bf16, tag="qtile")
                    ctxn0 = cxp.tile([P, DK, QT], bf16, tag="ctxn")
                    ctxn1 = cxp.tile([P, DK, QT], bf16, tag="ctxn")
                    nc.sync.dma_start(wv[:], wv_src)
                    nc.sync.dma_start(wot[:], wo_src)
                    # emission order = scheduler priority: qt0's pairs right
                    # after the k-tiles they need, so exp work is available
                    # while the rest of the K/V projection runs on the PE.
                    nt0 = (NKS[g][0] + 3) // 4   # x-tiles qt0 needs
                    nt1 = (NKS[g][1] + 3) // 4   # x-tiles qt1 needs (g1: 3)
                    with tc.tile_pool(name="wq", bufs=2) as wqp:
                        qproj(QSTART[g][0], qtile0, wqp)
                        kvproj(range(nt0))
                        for hp in range(H // 2):
                            attn_pair(NKS[g][0], qtile0, ctxn0, hp)
                        qproj(QSTART[g][1], qtile1, wqp)
                        kvproj(range(nt0, nt1))
                    for hp in range(H // 2):
                        attn_pair(NKS[g][1], qtile1, ctxn1, hp)
                    outproj_ln(0, ctxn0)
                    outproj_ln(1, ctxn1)

                pid = nc.partition_id()
                with tc.If(pid < 4) as cmp:
                    group(0)
                with cmp.Else():
                    group(1)
    nc.compile()
    return nc


def _get_nc():
    if "nc" not in _cache:
        _cache["nc"] = _build()
    return _cache["nc"]


def _prep(x, in_proj_w, in_proj_b, out_w, out_b, gamma, beta):
    from ml_dtypes import bfloat16

    x = np.asarray(x, np.float32)
    wtb = np.ascontiguousarray(np.asarray(in_proj_w, np.float32).T).astype(bfloat16)
    wob = np.ascontiguousarray(np.asarray(out_w, np.float32).T).astype(bfloat16)
    bqkv = np.asarray(in_proj_b, np.float32)
    bo = np.asarray(out_b, np.float32)
    gam = np.asarray(gamma, np.float32)
    bet = np.asarray(beta, np.float32)
    pp_, dd_, qq_ = np.arange(P)[:, None, None], np.arange(4)[None, :, None], \
        np.arange(QT)[None, None, :]
    maskd = (qq_ >= dd_ * P + pp_).astype(bfloat16)
    qcols = {
        0: np.r_[0:QT, 3 * QT:4 * QT],
        1: np.r_[QT:3 * QT],
    }
    in_maps = []
    for c in range(8):
        b, g = c % 4, c // 4
        xtb = np.ascontiguousarray(x[b].T).astype(bfloat16)
        xr = np.ascontiguousarray(x[b][qcols[g]]) + bo[None, :]
        in_maps.append({
            "xt": xtb,
            "xrd": xr.astype(np.float32),
            "wtb": wtb,
            "wob": wob,
            "mskd": maskd,
            "bqd": bqkv[0:D], "bkd": bqkv[D:2 * D],
            "bvd": bqkv[2 * D:3 * D].astype(bfloat16),
            "gamd": gam, "betd": bet,
        })
    return in_maps, qcols


def _run(in_maps, trace=False, **kw):
    from concourse.bass_utils import run_bass_kernel_spmd

    return run_bass_kernel_spmd(_get_nc(), in_maps, list(range(8)), trace=trace, **kw)


def kernel(x, in_proj_w, in_proj_b, out_w, out_b, gamma, beta):
    in_maps, qcols = _prep(x, in_proj_w, in_proj_b, out_w, out_b, gamma, beta)
    res = _run(in_maps)
    out = np.empty((B, S, D), np.float32)
    for c in range(8):
        out[c % 4, qcols[c // 4]] = res.results[c]["ytd"]
    return out


# revision 17
# speedup vs baseline: 1.3246x; 1.0834x over previous
"""Causal self-attention + residual + LayerNorm fused Trainium2 kernel.

Problem: B=4, S=2048, D=1024, H=16 heads (hd=64), fp32 in/out.
    qkv = x @ in_proj_w.T + in_proj_b ; causal MHA ; out proj ; y = LN(x + attn_out)

Sharding (zero cross-core communication, 8 NeuronCores):
    core c -> batch b = c % 4, query-group g = c // 4.
    Causal zig-zag balance: g=0 owns query blocks [0:512) and [1536:2048),
    g=1 owns [512:1536). Every core computes full K/V for its batch,
    attention + out-proj + residual + LayerNorm for its own queries.

Layout/precision strategy:
  - All matmul operands are bf16 (fp32 PSUM accumulation): same PE rate as
    fp32r but half the DMA/SBUF and 2x DVE modes.
  - K^T / Q^T computed transposed (features on partitions); V computed
    natural (tokens on partitions) with an augmented ones-column so the
    probability matmul also produces the softmax denominator in PSUM row 64.
    V carries its projection bias (the bias rides through softmax).
  - K/V/Q projections and attention are emitted inside the same If branch so
    the Tile scheduler overlaps the PE-heavy projections with the ACT-bound
    (exp) attention stream: attention on k-tile i only depends on the K/V
    tiles already produced.
  - Scores for a head pair go to [128, 2, 512] PSUM blocks; exp is one
    [128,1024] ACT instruction per block per head. The causal mask is a bf16
    multiply on the exp'd block (last two blocks only); the final block only
    computes its live query half.
  - Out-projection is token-oriented (queries on partitions): LayerNorm
    reduces along the free axis via bn_stats/bn_aggr and normalizes with
    per-partition tensor_scalar ops. out_b is folded into the residual on
    the host; gamma/beta applied via broadcast tiles built once at init.
"""
import sys

if "/opt/trn_rl_repo" not in sys.path:
    sys.path.insert(0, "/opt/trn_rl_repo")

import numpy as np

B, S, D, H, HD = 4, 2048, 1024, 16, 64
P = 128
QT = 512                       # queries per q-tile
NQ = 1024                      # queries per core
NKT = S // P                   # 16 k-tiles per batch
DK = D // P                    # 8 contraction tiles over D
QSTART = {0: (0, 1536), 1: (512, 1024)}   # group -> q-tile start columns
NKS = {0: (4, 16), 1: (8, 12)}            # group -> per-q-tile k-tile counts

_cache = {}


def _build():
    import concourse.mybir as mybir
    import concourse.tile as tile
    from concourse import bacc
    from concourse.bass import ts
    from concourse.alu_op_type import AluOpType

    f32 = mybir.dt.float32
    bf16 = mybir.dt.bfloat16
    AF = mybir.ActivationFunctionType

    nc = bacc.Bacc("TRN2", target_bir_lowering=False, debug=False, num_devices=8)

    xt = nc.dram_tensor("xt", [D, S], bf16, kind="ExternalInput").ap()
    xrd = nc.dram_tensor("xrd", [NQ, D], f32, kind="ExternalInput").ap()
    wtb = nc.dram_tensor("wtb", [D, 3 * D], bf16, kind="ExternalInput").ap()
    wob = nc.dram_tensor("wob", [D, D], bf16, kind="ExternalInput").ap()
    mskd = nc.dram_tensor("mskd", [P, 4, QT], bf16, kind="ExternalInput").ap()
    bqd = nc.dram_tensor("bqd", [D], f32, kind="ExternalInput").ap()
    bkd = nc.dram_tensor("bkd", [D], f32, kind="ExternalInput").ap()
    bvd = nc.dram_tensor("bvd", [D], bf16, kind="ExternalInput").ap()
    gamd = nc.dram_tensor("gamd", [D], f32, kind="ExternalInput").ap()
    betd = nc.dram_tensor("betd", [D], f32, kind="ExternalInput").ap()
    ytd = nc.dram_tensor("ytd", [NQ, D], f32, kind="ExternalOutput").ap()

    xt_r = xt.rearrange("(dk p) t -> p dk t", p=P)
    wq_src = wtb[:, 0:D].rearrange("(dk p) (f c) -> p dk f c", p=P, c=P)
    wk_src = wtb[:, D:2 * D].rearrange("(dk p) (f c) -> p dk f c", p=P, c=P)
    wv_src = wtb[:, 2 * D:3 * D].rearrange("(dk p) (g c) -> p dk g c", p=P, c=QT)
    wo_src = wob.rearrange("(dk p) f -> p dk f", p=P)

    with tile.TileContext(nc) as tc:
        with tc.tile_pool(name="persist", bufs=1) as pers:
            kt = pers.tile([P, DK, S], bf16)              # K^T      32 KB/part
            v = pers.tile([P, NKT, H, HD + 2], bf16)      # V aug    33 KB/part
            msk = pers.tile([P, 4, QT], bf16)
            bia = pers.tile([P, DK, 2], f32)              # bq bk per-partition
            gbc = pers.tile([P, D], f32)                 # gamma broadcast
            bbc = pers.tile([P, D], f32)                 # beta broadcast
            bvbc = pers.tile([P, 2, DK, HD], bf16)         # V-bias broadcast
            eps_t = pers.tile([P, 1], f32)

            nc.vector.memset(eps_t[:], 1e-5)
            nc.sync.dma_start(msk[:], mskd[:])
            nc.sync.dma_start(bia[:, :, 0], bqd.rearrange("(f p) -> p f", p=P))
            nc.sync.dma_start(bia[:, :, 1], bkd.rearrange("(f p) -> p f", p=P))
            nc.vector.memset(v[:, :, :, HD:HD + 1], 1.0)

            with (
                tc.tile_pool(name="initrows", bufs=1) as irp,
                tc.tile_pool(name="wk", bufs=2) as wkp,
                tc.tile_pool(name="wv", bufs=1) as wvp,
                tc.tile_pool(name="wot", bufs=1) as wotp,
                tc.tile_pool(name="xc", bufs=2) as xcp,
                tc.tile_pool(name="qt", bufs=2) as qtp,
                tc.tile_pool(name="cx", bufs=2) as cxp,
                tc.tile_pool(name="se", bufs=2) as sep,
                tc.tile_pool(name="scr", bufs=2) as scr,
                tc.tile_pool(name="yx", bufs=2) as yxp,
                tc.tile_pool(name="pp", bufs=2, space="PSUM") as pp,
                tc.tile_pool(name="s_ps", bufs=2, space="PSUM") as sps,
                tc.tile_pool(name="c_ps", bufs=2, space="PSUM") as cps,
            ):
                rowv = irp.tile([1, D], bf16)
                rowg = irp.tile([1, D], f32)
                rowb = irp.tile([1, D], f32)
                nc.sync.dma_start(rowv[:], bvd.rearrange("(a d) -> a d", a=1))
                nc.sync.dma_start(rowg[:], gamd.rearrange("(a d) -> a d", a=1))
                nc.sync.dma_start(rowb[:], betd.rearrange("(a d) -> a d", a=1))
                nc.gpsimd.partition_broadcast(bvbc[:], rowv[:])
                nc.gpsimd.partition_broadcast(gbc[:], rowg[:])
                nc.gpsimd.partition_broadcast(bbc[:], rowb[:])

                wv = wvp.tile([P, DK, 2, QT], bf16)
                wot = wotp.tile([P, DK, D], bf16)

                def kvproj(trange):
                    for t in trange:
                        xc = xcp.tile([P, DK, QT], bf16, tag="xc")
                        nc.sync.dma_start(xc[:], xt_r[:, :, ts(t, QT)])
                        for f in range(DK):
                            wkt = wkp.tile([P, DK, P], bf16, tag="wk")
                            nc.sync.dma_start(wkt[:], wk_src[:, :, f, :])
                            ps = pp.tile([P, QT], f32, tag="pp")
                            for dk in range(DK):
                                nc.tensor.matmul(
                                    ps[:], wkt[:, dk, :], xc[:, dk, :],
                                    start=(dk == 0), stop=(dk == DK - 1),
                                )
                            nc.vector.tensor_scalar_add(
                                kt[:, f, ts(t, QT)], ps[:], bia[:, f, 1:2])
                        for q4 in range(4):
                            kti = 4 * t + q4
                            for fg in range(2):
                                ps = pp.tile([P, DK, HD], f32, tag="pp")
                                for dk in range(DK):
                                    nc.tensor.matmul(
                                        ps[:], xc[:, dk, ts(q4, P)],
                                        wv[:, dk, fg, :],
                                        start=(dk == 0), stop=(dk == DK - 1),
                                    )
                                nc.vector.tensor_add(
                                    v[:, kti, 8 * fg:8 * fg + 8, 0:HD],
                                    ps[:], bvbc[:, fg, :, :],
                                )

                def qproj(qpos, qtile, wqp):
                    xcq = xcp.tile([P, DK, QT], bf16, tag="xc")
                    nc.sync.dma_start(xcq[:], xt_r[:, :, qpos:qpos + QT])
                    for f in range(DK):
                        wq = wqp.tile([P, DK, P], bf16, tag="wq")
                        nc.sync.dma_start(wq[:], wq_src[:, :, f, :])
                        ps = pp.tile([P, QT], f32, tag="pp")
                        for dk in range(DK):
                            nc.tensor.matmul(
                                ps[:], wq[:, dk, :], xcq[:, dk, :],
                                start=(dk == 0), stop=(dk == DK - 1),
                            )
                        nc.vector.tensor_scalar_add(
                            qtile[:, f, :], ps[:], bia[:, f, 0:1])

                def attn_pair(nk, qtile, ctxn, hp):
                    nblk = nk // 2
                    if True:
                        cp0 = cps.tile([HD + 1, QT], f32, tag="c")
                        cp1 = cps.tile([HD + 1, QT], f32, tag="c")
                        for blk in range(nblk):
                            i0 = 2 * blk
                            # final diagonal block: only queries >= 256 have
                            # any live key; compute the live half only
                            qs_ = slice(QT // 2, QT) if blk == nblk - 1 \
                                else slice(0, QT)
                            sp0 = sps.tile([P, 2, QT], f32, tag="s")
                            sp1 = sps.tile([P, 2, QT], f32, tag="s")
                            for j in range(2):
                                nc.tensor.matmul(
                                    sp0[:, j, qs_], kt[0:HD, hp, ts(i0 + j, P)],
                                    qtile[0:HD, hp, qs_], start=True, stop=True,
                                )
                                nc.tensor.matmul(
                                    sp1[:, j, qs_], kt[HD:P, hp, ts(i0 + j, P)],
                                    qtile[HD:P, hp, qs_], start=True, stop=True,
                                )
                            se0 = sep.tile([P, 2, QT], bf16, tag="se")
                            se1 = sep.tile([P, 2, QT], bf16, tag="se")
                            nc.scalar.activation(
                                se0[:, :, qs_], sp0[:, :, qs_], AF.Exp, scale=0.125)
                            nc.scalar.activation(
                                se1[:, :, qs_], sp1[:, :, qs_], AF.Exp, scale=0.125)
                            if blk >= nblk - 2:
                                dd = 2 * blk - (nk - 4)
                                nc.vector.tensor_mul(
                                    se0[:, :, qs_], se0[:, :, qs_],
                                    msk[:, dd:dd + 2, qs_])
                                nc.vector.tensor_mul(
                                    se1[:, :, qs_], se1[:, :, qs_],
                                    msk[:, dd:dd + 2, qs_])
                            for j in range(2):
                                st = (blk == 0 and j == 0)
                                sp_ = (blk == nblk - 1 and j == 1)
                                nc.tensor.matmul(
                                    cp0[:, qs_], v[:, i0 + j, 2 * hp, 0:HD + 1],
                                    se0[:, j, qs_], start=st, stop=sp_,
                                )
                                nc.tensor.matmul(
                                    cp1[:, qs_], v[:, i0 + j, 2 * hp + 1, 0:HD + 1],
                                    se1[:, j, qs_], start=st, stop=sp_,
                                )
                        # stage ctx+denominator to SBUF immediately: the
                        # two copies free the PSUM banks in ~1.5us so the
                        # next pair's accumulation never waits on the
                        # normalization chain below.
                        cu = scr.tile([HD + 1, 2, QT], bf16, tag="cu")
                        nc.vector.tensor_copy(cu[:, 0, :], cp0[:])
                        nc.vector.tensor_copy(cu[:, 1, :], cp1[:])
                        # denominators on partition 0 (engine APs may only
                        # start at quadrant partition bases), fp32 for recip
                        den2 = scr.tile([1, 2, QT], f32, tag="den", bufs=1)
                        nc.vector.tensor_copy(den2[:, 0, :], cp0[HD:HD + 1, :])
                        nc.vector.tensor_copy(den2[:, 1, :], cp1[HD:HD + 1, :])
                        rec2 = scr.tile([1, 2, QT], f32, tag="rec", bufs=1)
                        nc.vector.reciprocal_approx_fast(out=rec2[:], in_=den2[:])
                        rec2b = scr.tile([1, 2, QT], bf16, tag="recb", bufs=1)
                        nc.vector.tensor_copy(rec2b[:], rec2[:])
                        bc2 = scr.tile([HD, 2, QT], bf16, tag="bc", bufs=1)
                        nc.gpsimd.partition_broadcast(bc2[:], rec2b[:])
                        nc.vector.tensor_mul(
                            ctxn[0:HD, hp, :], cu[0:HD, 0, :], bc2[:, 0, :])
                        nc.vector.tensor_mul(
                            ctxn[HD:P, hp, :], cu[0:HD, 1, :], bc2[:, 1, :])

                def outproj_ln(qt, ctxn):
                    for qs in range(4):
                        row0 = qt * QT + qs * P
                        xrt = yxp.tile([P, D], f32, tag="xr")
                        nc.sync.dma_start(xrt[:], xrd[row0:row0 + P, :])
                        for fh in range(2):
                            ps = pp.tile([P, QT], f32, tag="pp")
                            for dk in range(DK):
                                nc.tensor.matmul(
                                    ps[:], ctxn[:, dk, ts(qs, P)],
                                    wot[:, dk, ts(fh, QT)],
                                    start=(dk == 0), stop=(dk == DK - 1),
                                )
                            nc.vector.tensor_add(
                                xrt[:, ts(fh, QT)], ps[:], xrt[:, ts(fh, QT)])
                        st6 = scr.tile([P, 12], f32, tag="st6")
                        nc.vector.bn_stats(st6[:, 0:6], xrt[:, 0:QT])
                        nc.vector.bn_stats(st6[:, 6:12], xrt[:, QT:D])
                        mv = scr.tile([P, 2], f32, tag="mv")
                        nc.vector.bn_aggr(mv[:], st6[:])
                        sd = scr.tile([P, 1], f32, tag="sd")
                        nc.scalar.activation(
                            sd[:], mv[:, 1:2], AF.Sqrt, bias=eps_t[:])
                        rstd = scr.tile([P, 1], f32, tag="rstd")
                        rss = scr.tile([P, 1], f32, tag="rss")
                        nc.vector.reciprocal_approx_accurate(
                            rstd[:], sd[:], rss[:])
                        nc.vector.tensor_scalar(
                            xrt[:], xrt[:], mv[:, 0:1], rstd[:],
                            AluOpType.subtract, AluOpType.mult,
                        )
                        nc.gpsimd.tensor_mul(xrt[:], xrt[:], gbc[:])
                        nc.gpsimd.tensor_add(xrt[:], xrt[:], bbc[:])
                        nc.sync.dma_start(ytd[row0:row0 + P, :], xrt[:])

                def group(g):
                    qtile0 = qtp.tile([P, DK, QT], bf16, tag="qtile")
                    qtile1 = qtp.tile([P, DK, QT], bf16, tag="qtile")
                    ctxn0 = cxp.tile([P, DK, QT], bf16, tag="ctxn")
                    ctxn1 = cxp.tile([P, DK, QT], bf16, tag="ctxn")
                    nc.sync.dma_start(wv[:], wv_src)
                    nc.sync.dma_start(wot[:], wo_src)
                    # emission order = scheduler priority: qt0's pairs right
                    # after the k-tiles they need, so exp work is available
                    # while the rest of the K/V projection runs on the PE.
                    nt0 = (NKS[g][0] + 3) // 4   # x-tiles qt0 needs
                    nt1 = (NKS[g][1] + 3) // 4   # x-tiles qt1 needs (g1: 3)
                    with tc.tile_pool(name="wq", bufs=2) as wqp:
                        qproj(QSTART[g][0], qtile0, wqp)
                        kvproj(range(nt0))
                        for hp in range(H // 2):
                            attn_pair(NKS[g][0], qtile0, ctxn0, hp)
                        qproj(QSTART[g][1], qtile1, wqp)
                        kvproj(range(nt0, nt1))
                    for hp in range(H // 2):
                        attn_pair(NKS[g][1], qtile1, ctxn1, hp)
                    outproj_ln(0, ctxn0)
                    outproj_ln(1, ctxn1)

                pid = nc.partition_id()
                with tc.If(pid < 4) as cmp:
                    group(0)
                with cmp.Else():
                    group(1)
    nc.compile()
    return nc


def _get_nc():
    if "nc" not in _cache:
        _cache["nc"] = _build()
    return _cache["nc"]


def _prep(x, in_proj_w, in_proj_b, out_w, out_b, gamma, beta):
    from ml_dtypes import bfloat16

    x = np.asarray(x, np.float32)
    wtb = np.ascontiguousarray(np.asarray(in_proj_w, np.float32).T).astype(bfloat16)
    wob = np.ascontiguousarray(np.asarray(out_w, np.float32).T).astype(bfloat16)
    bqkv = np.asarray(in_proj_b, np.float32)
    bo = np.asarray(out_b, np.float32)
    gam = np.asarray(gamma, np.float32)
    bet = np.asarray(beta, np.float32)
    pp_, dd_, qq_ = np.arange(P)[:, None, None], np.arange(4)[None, :, None], \
        np.arange(QT)[None, None, :]
    maskd = (qq_ >= dd_ * P + pp_).astype(bfloat16)
    qcols = {
        0: np.r_[0:QT, 3 * QT:4 * QT],
        1: np.r_[QT:3 * QT],
    }
    in_maps = []
    for c in range(8):
        b, g = c % 4, c // 4
        xtb = np.ascontiguousarray(x[b].T).astype(bfloat16)
        xr = np.ascontiguousarray(x[b][qcols[g]]) + bo[None, :]
        in_maps.append({
            "xt": xtb,
            "xrd": xr.astype(np.float32),
            "wtb": wtb,
            "wob": wob,
            "mskd": maskd,
            "bqd": bqkv[0:D], "bkd": bqkv[D:2 * D],
            "bvd": bqkv[2 * D:3 * D].astype(bfloat16),
            "gamd": gam, "betd": bet,
        })
    return in_maps, qcols


def _run(in_maps, trace=False, **kw):
    from concourse.bass_utils import run_bass_kernel_spmd

    return run_bass_kernel_spmd(_get_nc(), in_maps, list(range(8)), trace=trace, **kw)


def kernel(x, in_proj_w, in_proj_b, out_w, out_b, gamma, beta):
    in_maps, qcols = _prep(x, in_proj_w, in_proj_b, out_w, out_b, gamma, beta)
    res = _run(in_maps)
    out = np.empty((B, S, D), np.float32)
    for c in range(8):
        out[c % 4, qcols[c // 4]] = res.results[c]["ytd"]
    return out


# revision 19
# speedup vs baseline: 1.3366x; 1.0090x over previous
"""Causal self-attention + residual + LayerNorm fused Trainium2 kernel.

Problem: B=4, S=2048, D=1024, H=16 heads (hd=64), fp32 in/out.
    qkv = x @ in_proj_w.T + in_proj_b ; causal MHA ; out proj ; y = LN(x + attn_out)

Sharding (zero cross-core communication, 8 NeuronCores):
    core c -> batch b = c % 4, query-group g = c // 4.
    Causal zig-zag balance: g=0 owns query blocks [0:512) and [1536:2048),
    g=1 owns [512:1536). Every core computes full K/V for its batch,
    attention + out-proj + residual + LayerNorm for its own queries.

Layout/precision strategy:
  - All matmul operands are bf16 (fp32 PSUM accumulation): same PE rate as
    fp32r but half the DMA/SBUF and 2x DVE modes.
  - K^T / Q^T computed transposed (features on partitions); V computed
    natural (tokens on partitions) with an augmented ones-column so the
    probability matmul also produces the softmax denominator in PSUM row 64.
    V carries its projection bias (the bias rides through softmax).
  - K/V/Q projections and attention are emitted inside the same If branch so
    the Tile scheduler overlaps the PE-heavy projections with the ACT-bound
    (exp) attention stream: attention on k-tile i only depends on the K/V
    tiles already produced.
  - Scores for a head pair go to [128, 2, 512] PSUM blocks; exp is one
    [128,1024] ACT instruction per block per head. The causal mask is a bf16
    multiply on the exp'd block (last two blocks only); the final block only
    computes its live query half.
  - Out-projection is token-oriented (queries on partitions): LayerNorm
    reduces along the free axis via bn_stats/bn_aggr and normalizes with
    per-partition tensor_scalar ops. out_b is folded into the residual on
    the host; gamma/beta applied via broadcast tiles built once at init.
"""
import sys

if "/opt/trn_rl_repo" not in sys.path:
    sys.path.insert(0, "/opt/trn_rl_repo")

import numpy as np

B, S, D, H, HD = 4, 2048, 1024, 16, 64
P = 128
QT = 512                       # queries per q-tile
NQ = 1024                      # queries per core
NKT = S // P                   # 16 k-tiles per batch
DK = D // P                    # 8 contraction tiles over D
QSTART = {0: (0, 1536), 1: (512, 1024)}   # group -> q-tile start columns
NKS = {0: (4, 16), 1: (8, 12)}            # group -> per-q-tile k-tile counts

_cache = {}


def _build():
    import concourse.mybir as mybir
    import concourse.tile as tile
    from concourse import bacc
    from concourse.bass import ts
    from concourse.alu_op_type import AluOpType

    f32 = mybir.dt.float32
    bf16 = mybir.dt.bfloat16
    AF = mybir.ActivationFunctionType

    nc = bacc.Bacc("TRN2", target_bir_lowering=False, debug=False, num_devices=8)

    xt = nc.dram_tensor("xt", [D, S], bf16, kind="ExternalInput").ap()
    xrd = nc.dram_tensor("xrd", [NQ, D], f32, kind="ExternalInput").ap()
    wtb = nc.dram_tensor("wtb", [D, 3 * D], bf16, kind="ExternalInput").ap()
    wob = nc.dram_tensor("wob", [D, D], bf16, kind="ExternalInput").ap()
    mskd = nc.dram_tensor("mskd", [P, 4, QT], bf16, kind="ExternalInput").ap()
    bqkv = nc.dram_tensor("bqkv", [D, 3], f32, kind="ExternalInput").ap()
    gbd = nc.dram_tensor("gbd", [2, D], f32, kind="ExternalInput").ap()
    ytd = nc.dram_tensor("ytd", [NQ, D], f32, kind="ExternalOutput").ap()

    xt_r = xt.rearrange("(dk p) t -> p dk t", p=P)
    wq_src = wtb[:, 0:D].rearrange("(dk p) (f c) -> p dk f c", p=P, c=P)
    wk_src = wtb[:, D:2 * D].rearrange("(dk p) (f c) -> p dk f c", p=P, c=P)
    wv_src = wtb[:, 2 * D:3 * D].rearrange("(dk p) (g c) -> p dk g c", p=P, c=QT)
    wo_src = wob.rearrange("(dk p) f -> p dk f", p=P)

    with tile.TileContext(nc) as tc:
        with tc.tile_pool(name="persist", bufs=1) as pers:
            kt = pers.tile([P, DK, S], bf16)              # K^T      32 KB/part
            v = pers.tile([P, NKT, H, HD + 2], bf16)      # V aug    33 KB/part
            msk = pers.tile([P, 4, QT], bf16)
            bia = pers.tile([P, DK, 3], f32)              # bq bk bv per-part
            gb2 = pers.tile([P, 2, D], f32)              # gamma/beta bcast
            bvbc = pers.tile([P, 2, DK, HD], bf16)         # V-bias broadcast
            eps_t = pers.tile([P, 1], f32)

            nc.vector.memset(eps_t[:], 1e-5)
            nc.sync.dma_start(msk[:], mskd[:])
            nc.sync.dma_start(
                bia[:], bqkv.rearrange("(f p) c -> p f c", p=P))
            nc.vector.memset(v[:, :, :, HD:HD + 1], 1.0)

            with (
                tc.tile_pool(name="initrows", bufs=1) as irp,
                tc.tile_pool(name="wk", bufs=2) as wkp,
                tc.tile_pool(name="wv", bufs=1) as wvp,
                tc.tile_pool(name="wot", bufs=1) as wotp,
                tc.tile_pool(name="xc", bufs=2) as xcp,
                tc.tile_pool(name="qt", bufs=2) as qtp,
                tc.tile_pool(name="cx", bufs=2) as cxp,
                tc.tile_pool(name="se", bufs=2) as sep,
                tc.tile_pool(name="scr", bufs=2) as scr,
                tc.tile_pool(name="yx", bufs=2) as yxp,
                tc.tile_pool(name="pp", bufs=2, space="PSUM") as pp,
                tc.tile_pool(name="s_ps", bufs=2, space="PSUM") as sps,
                tc.tile_pool(name="c_ps", bufs=2, space="PSUM") as cps,
            ):
                rowgb = irp.tile([1, 2, D], f32)
                nc.sync.dma_start(rowgb[:], gbd.rearrange("(a c) d -> a c d", a=1))
                nc.gpsimd.partition_broadcast(gb2[:], rowgb[:])

                wv = wvp.tile([P, DK, 2, QT], bf16)
                wot = wotp.tile([P, DK, D], bf16)

                def kvproj(trange):
                    for t in trange:
                        xc = xcp.tile([P, DK, QT], bf16, tag="xc")
                        nc.sync.dma_start(xc[:], xt_r[:, :, ts(t, QT)])
                        for f in range(DK):
                            wkt = wkp.tile([P, DK, P], bf16, tag="wk")
                            nc.sync.dma_start(wkt[:], wk_src[:, :, f, :])
                            ps = pp.tile([P, QT], f32, tag="pp")
                            for dk in range(DK):
                                nc.tensor.matmul(
                                    ps[:], wkt[:, dk, :], xc[:, dk, :],
                                    start=(dk == 0), stop=(dk == DK - 1),
                                )
                            nc.vector.tensor_scalar_add(
                                kt[:, f, ts(t, QT)], ps[:], bia[:, f, 1:2])
                        for q4 in range(4):
                            kti = 4 * t + q4
                            for fg in range(2):
                                ps = pp.tile([P, DK, HD], f32, tag="pp")
                                for dk in range(DK):
                                    nc.tensor.matmul(
                                        ps[:], xc[:, dk, ts(q4, P)],
                                        wv[:, dk, fg, :],
                                        start=(dk == 0), stop=(dk == DK - 1),
                                    )
                                nc.vector.tensor_copy(
                                    v[:, kti, 8 * fg:8 * fg + 8, 0:HD], ps[:])

                def qproj(qpos, qtile, wqp):
                    xcq = xcp.tile([P, DK, QT], bf16, tag="xc")
                    nc.sync.dma_start(xcq[:], xt_r[:, :, qpos:qpos + QT])
                    for f in range(DK):
                        wq = wqp.tile([P, DK, P], bf16, tag="wq")
                        nc.sync.dma_start(wq[:], wq_src[:, :, f, :])
                        ps = pp.tile([P, QT], f32, tag="pp")
                        for dk in range(DK):
                            nc.tensor.matmul(
                                ps[:], wq[:, dk, :], xcq[:, dk, :],
                                start=(dk == 0), stop=(dk == DK - 1),
                            )
                        nc.vector.tensor_scalar_add(
                            qtile[:, f, :], ps[:], bia[:, f, 0:1])

                def attn_pair(nk, qtile, ctxn, hp):
                    nblk = nk // 2
                    if True:
                        cp0 = cps.tile([HD + 1, QT], f32, tag="c")
                        cp1 = cps.tile([HD + 1, QT], f32, tag="c")
                        for blk in range(nblk):
                            i0 = 2 * blk
                            # final diagonal block: only queries >= 256 have
                            # any live key; compute the live half only
                            qs_ = slice(QT // 2, QT) if blk == nblk - 1 \
                                else slice(0, QT)
                            sp0 = sps.tile([P, 2, QT], f32, tag="s")
                            sp1 = sps.tile([P, 2, QT], f32, tag="s")
                            for j in range(2):
                                nc.tensor.matmul(
                                    sp0[:, j, qs_], kt[0:HD, hp, ts(i0 + j, P)],
                                    qtile[0:HD, hp, qs_], start=True, stop=True,
                                )
                                nc.tensor.matmul(
                                    sp1[:, j, qs_], kt[HD:P, hp, ts(i0 + j, P)],
                                    qtile[HD:P, hp, qs_], start=True, stop=True,
                                )
                            se0 = sep.tile([P, 2, QT], bf16, tag="se")
                            se1 = sep.tile([P, 2, QT], bf16, tag="se")
                            nc.scalar.activation(
                                se0[:, :, qs_], sp0[:, :, qs_], AF.Exp, scale=0.125)
                            nc.scalar.activation(
                                se1[:, :, qs_], sp1[:, :, qs_], AF.Exp, scale=0.125)
                            if blk >= nblk - 2:
                                dd = 2 * blk - (nk - 4)
                                nc.vector.tensor_mul(
                                    se0[:, :, qs_], se0[:, :, qs_],
                                    msk[:, dd:dd + 2, qs_])
                                nc.vector.tensor_mul(
                                    se1[:, :, qs_], se1[:, :, qs_],
                                    msk[:, dd:dd + 2, qs_])
                            for j in range(2):
                                st = (blk == 0 and j == 0)
                                sp_ = (blk == nblk - 1 and j == 1)
                                nc.tensor.matmul(
                                    cp0[:, qs_], v[:, i0 + j, 2 * hp, 0:HD + 1],
                                    se0[:, j, qs_], start=st, stop=sp_,
                                )
                                nc.tensor.matmul(
                                    cp1[:, qs_], v[:, i0 + j, 2 * hp + 1, 0:HD + 1],
                                    se1[:, j, qs_], start=st, stop=sp_,
                                )
                        # stage ctx+denominator to SBUF immediately: the
                        # two copies free the PSUM banks in ~1.5us so the
                        # next pair's accumulation never waits on the
                        # normalization chain below.
                        cu = scr.tile([HD + 1, 2, QT], bf16, tag="cu")
                        nc.vector.tensor_copy(cu[:, 0, :], cp0[:])
                        nc.vector.tensor_copy(cu[:, 1, :], cp1[:])
                        # denominators on partition 0 (engine APs may only
                        # start at quadrant partition bases), fp32 for recip
                        den2 = scr.tile([1, 2, QT], f32, tag="den", bufs=1)
                        nc.vector.tensor_copy(den2[:, 0, :], cp0[HD:HD + 1, :])
                        nc.vector.tensor_copy(den2[:, 1, :], cp1[HD:HD + 1, :])
                        rec2 = scr.tile([1, 2, QT], f32, tag="rec", bufs=1)
                        nc.vector.reciprocal_approx_fast(out=rec2[:], in_=den2[:])
                        rec2b = scr.tile([1, 2, QT], bf16, tag="recb", bufs=1)
                        nc.vector.tensor_copy(rec2b[:], rec2[:])
                        bc2 = scr.tile([HD, 2, QT], bf16, tag="bc", bufs=1)
                        nc.gpsimd.partition_broadcast(bc2[:], rec2b[:])
                        nc.vector.tensor_mul(
                            ctxn[0:HD, hp, :], cu[0:HD, 0, :], bc2[:, 0, :])
                        nc.vector.tensor_mul(
                            ctxn[HD:P, hp, :], cu[0:HD, 1, :], bc2[:, 1, :])
                        nc.vector.tensor_scalar_add(
                            ctxn[0:HD, hp, :], ctxn[0:HD, hp, :],
                            bia[0:HD, hp, 2:3])
                        nc.vector.tensor_scalar_add(
                            ctxn[HD:P, hp, :], ctxn[HD:P, hp, :],
                            bia[HD:P, hp, 2:3])

                def outproj_ln(qt, ctxn):
                    for qs in range(4):
                        row0 = qt * QT + qs * P
                        xrt = yxp.tile([P, D], f32, tag="xr")
                        nc.sync.dma_start(xrt[:], xrd[row0:row0 + P, :])
                        for fh in range(2):
                            ps = pp.tile([P, QT], f32, tag="pp")
                            for dk in range(DK):
                                nc.tensor.matmul(
                                    ps[:], ctxn[:, dk, ts(qs, P)],
                                    wot[:, dk, ts(fh, QT)],
                                    start=(dk == 0), stop=(dk == DK - 1),
                                )
                            nc.vector.tensor_add(
                                xrt[:, ts(fh, QT)], ps[:], xrt[:, ts(fh, QT)])
                        st6 = scr.tile([P, 12], f32, tag="st6")
                        nc.vector.bn_stats(st6[:, 0:6], xrt[:, 0:QT])
                        nc.vector.bn_stats(st6[:, 6:12], xrt[:, QT:D])
                        mv = scr.tile([P, 2], f32, tag="mv")
                        nc.vector.bn_aggr(mv[:], st6[:])
                        lv = scr.tile([P, 1], f32, tag="lv")
                        nc.scalar.activation(
                            lv[:], mv[:, 1:2], AF.Ln, bias=eps_t[:])
                        rstd = scr.tile([P, 1], f32, tag="rstd")
                        nc.scalar.activation(rstd[:], lv[:], AF.Exp, scale=-0.5)
                        nc.vector.tensor_scalar(
                            xrt[:], xrt[:], mv[:, 0:1], rstd[:],
                            AluOpType.subtract, AluOpType.mult,
                        )
                        nc.gpsimd.tensor_mul(xrt[:], xrt[:], gb2[:, 0, :])
                        nc.gpsimd.tensor_add(xrt[:], xrt[:], gb2[:, 1, :])
                        nc.sync.dma_start(ytd[row0:row0 + P, :], xrt[:])

                def group(g):
                    qtile0 = qtp.tile([P, DK, QT], bf16, tag="qtile")
                    qtile1 = qtp.tile([P, DK, QT], bf16, tag="qtile")
                    ctxn0 = cxp.tile([P, DK, QT], bf16, tag="ctxn")
                    ctxn1 = cxp.tile([P, DK, QT], bf16, tag="ctxn")
                    nc.sync.dma_start(wv[:], wv_src)
                    nc.sync.dma_start(wot[:], wo_src)
                    # emission order = scheduler priority: qt0's pairs right
                    # after the k-tiles they need, so exp work is available
                    # while the rest of the K/V projection runs on the PE.
                    nt0 = (NKS[g][0] + 3) // 4   # x-tiles qt0 needs
                    nt1 = (NKS[g][1] + 3) // 4   # x-tiles qt1 needs (g1: 3)
                    with tc.tile_pool(name="wq", bufs=2) as wqp:
                        qproj(QSTART[g][0], qtile0, wqp)
                        kvproj(range(nt0))
                        for hp in range(H // 2):
                            attn_pair(NKS[g][0], qtile0, ctxn0, hp)
                        qproj(QSTART[g][1], qtile1, wqp)
                        kvproj(range(nt0, nt1))
                    for hp in range(H // 2):
                        attn_pair(NKS[g][1], qtile1, ctxn1, hp)
                    outproj_ln(0, ctxn0)
                    outproj_ln(1, ctxn1)

                pid = nc.partition_id()
                with tc.If(pid < 4) as cmp:
                    group(0)
                with cmp.Else():
                    group(1)
    nc.compile()
    return nc


def _get_nc():
    if "nc" not in _cache:
        _cache["nc"] = _build()
    return _cache["nc"]


def _prep(x, in_proj_w, in_proj_b, out_w, out_b, gamma, beta):
    from ml_dtypes import bfloat16

    x = np.asarray(x, np.float32)
    wtb = np.ascontiguousarray(np.asarray(in_proj_w, np.float32).T).astype(bfloat16)
    wob = np.ascontiguousarray(np.asarray(out_w, np.float32).T).astype(bfloat16)
    bqkv = np.asarray(in_proj_b, np.float32)
    bo = np.asarray(out_b, np.float32)
    gam = np.asarray(gamma, np.float32)
    bet = np.asarray(beta, np.float32)
    pp_, dd_, qq_ = np.arange(P)[:, None, None], np.arange(4)[None, :, None], \
        np.arange(QT)[None, None, :]
    maskd = (qq_ >= dd_ * P + pp_).astype(bfloat16)
    qcols = {
        0: np.r_[0:QT, 3 * QT:4 * QT],
        1: np.r_[QT:3 * QT],
    }
    in_maps = []
    for c in range(8):
        b, g = c % 4, c // 4
        xtb = np.ascontiguousarray(x[b].T).astype(bfloat16)
        xr = np.ascontiguousarray(x[b][qcols[g]]) + bo[None, :]
        in_maps.append({
            "xt": xtb,
            "xrd": xr.astype(np.float32),
            "wtb": wtb,
            "wob": wob,
            "mskd": maskd,
            "bqkv": np.ascontiguousarray(
                np.stack([bqkv[0:D], bqkv[D:2 * D], bqkv[2 * D:3 * D]], 1)),
            "gbd": np.ascontiguousarray(np.stack([gam, bet], 0)),
        })
    return in_maps, qcols


def _run(in_maps, trace=False, **kw):
    from concourse.bass_utils import run_bass_kernel_spmd

    return run_bass_kernel_spmd(_get_nc(), in_maps, list(range(8)), trace=trace, **kw)


def kernel(x, in_proj_w, in_proj_b, out_w, out_b, gamma, beta):
    in_maps, qcols = _prep(x, in_proj_w, in_proj_b, out_w, out_b, gamma, beta)
    res = _run(in_maps)
    out = np.empty((B, S, D), np.float32)
    for c in range(8):
        out[c % 4, qcols[c // 4]] = res.results[c]["ytd"]
    return out
